# revision 13
# baseline (speedup 1.0000x reference)
"""Trainium2 Bass kernel for nn_ChannelAttnBlock (GroupNorm + channel attention).

Self-contained: takes FULL unsharded inputs, shards batch over 8 NeuronCores
(2 batches/core), runs one SPMD NEFF, gathers the full output.

Per-core dataflow (B=2 batches, C=512 channels, T=8192), v2:
  pass 0: stream x (f32, single HBM read), cast to bf16 SBUF cache (gpsimd),
          bn_stats -> per-channel mean/var; selector matmuls aggregate the
          32 GN groups -> per-channel affine a, b. GN affine is folded into
          the q/k weights (wqf = a*wqt, bf16) and bias rows cq = Wq b + bq.
  pass A: qT/kT = x^T @ wqf + cq (t on partitions, bf16 matmuls; channel
          softmax is a free-dim segment reduce); exp on ACT (bf16 out);
          softmax denominators folded into kp = ek/(Sq*Sk) (bf16);
          w accumulated in PSUM as per-128-block outer products over t.
  M-fuse: h2 = BD(w) v and out-proj collapse into a single matrix:
          out = x + M''^T(ish) where M = Wp*BD(w)*Wv, M'' = M*diag(a),
          cP = M b (+ Wp BD(w) bv + bp). Computed on-chip per batch with
          a handful of small matmuls (no transposes needed: MT computed
          directly as A^T-free chain).
  pass B: out = x + M''x + cP: 16 bf16 matmuls per macro from the bf16
          x cache; residual+bias via gpsimd STT; bf16 out written to HBM.
"""

import numpy as np

C = 512
NH = 16      # heads
HC = 32      # channels/head
G = 32       # groupnorm groups
CG = C // G  # 16 channels per group
EPS = 1e-6

N_CORES = 8
B_FULL = 16
T_FULL = 8192
B_SHARD = B_FULL // N_CORES  # 2
TT = 512                     # t macro-tile
NM = T_FULL // TT            # 16 macros per batch


def build_nc(B, T, has_qk_bias=True, has_bv=True, debug=False):
    import concourse.tile as tile
    import concourse.mybir as mybir
    from concourse import bacc

    NMi = T // TT
    f32 = mybir.dt.float32
    bf16 = mybir.dt.bfloat16
    AF = mybir.ActivationFunctionType
    ALU = mybir.AluOpType
    AX = mybir.AxisListType

    nc = bacc.Bacc("TRN2", target_bir_lowering=False, debug=debug)

    x_d = nc.dram_tensor("x", [B, C, T], f32, kind="ExternalInput").ap()
    wqt_d = nc.dram_tensor("wqt", [C, C], bf16, kind="ExternalInput").ap()
    wkt_d = nc.dram_tensor("wkt", [C, C], bf16, kind="ExternalInput").ap()
    wv_d = nc.dram_tensor("wv", [C, C], bf16, kind="ExternalInput").ap()
    wpt_d = nc.dram_tensor("wpt", [C, C], bf16, kind="ExternalInput").ap()
    gammaP_d = nc.dram_tensor("gammaP", [128, 4], f32, kind="ExternalInput").ap()
    betaP_d = nc.dram_tensor("betaP", [128, 4], f32, kind="ExternalInput").ap()
    if has_qk_bias:
        bq_row_d = nc.dram_tensor("bq_row", [1, C], f32,
                                  kind="ExternalInput").ap()
        bk_row_d = nc.dram_tensor("bk_row", [1, C], f32,
                                  kind="ExternalInput").ap()
    bvP_d = nc.dram_tensor("bvP", [128, 4], f32, kind="ExternalInput").ap()
    bpP_d = nc.dram_tensor("bpP", [128, 4], f32, kind="ExternalInput").ap()
    sel_d = nc.dram_tensor("sel", [128, 8], f32, kind="ExternalInput").ap()
    selT_d = nc.dram_tensor("selT", [8, 128], f32, kind="ExternalInput").ap()
    ones1_d = nc.dram_tensor("ones1", [1, 128], bf16, kind="ExternalInput").ap()
    maskh_d = nc.dram_tensor("maskh", [128, 128], bf16, kind="ExternalInput").ap()
    out_d = nc.dram_tensor("out", [B, C, T], bf16, kind="ExternalOutput").ap()

    from contextlib import ExitStack

    with tile.TileContext(nc) as tc, ExitStack() as est:
        p = lambda name, bufs: est.enter_context(
            tc.tile_pool(name=name, bufs=bufs))
        wpool = p("wpool", 1)
        cpool = p("cpool", 1)
        xcache = p("xcache", 2)
        xin = p("xin", 2)
        stpool = p("stpool", 2)
        wfpool = p("wfpool", 1)
        cqpool = p("cqpool", 1)
        eqpool = p("eqpool", 2)
        ekpool = p("ekpool", 1)
        kppool = p("kppool", 2)
        smpool = p("smpool", 4)
        wsbpool = p("wsbpool", 1)
        mpool = p("mpool", 1)
        opool = p("opool", 2)

        # ---- load weights & constants ----
        wqt_sb = wpool.tile([128, 4, C], bf16)
        wkt_sb = wpool.tile([128, 4, C], bf16)
        wv_sb = wpool.tile([128, 4, C], bf16)
        wpt_sb = wpool.tile([128, 4, C], bf16)
        for j in range(4):
            nc.sync.dma_start(wqt_sb[:, j, :], wqt_d[128 * j:128 * j + 128, :])
            nc.sync.dma_start(wkt_sb[:, j, :], wkt_d[128 * j:128 * j + 128, :])
            nc.sync.dma_start(wv_sb[:, j, :], wv_d[128 * j:128 * j + 128, :])
            nc.sync.dma_start(wpt_sb[:, j, :], wpt_d[128 * j:128 * j + 128, :])
        gammaP = cpool.tile([128, 4], f32)
        betaP = cpool.tile([128, 4], f32)
        bvP = cpool.tile([128, 4], f32)
        bpP = cpool.tile([128, 4], f32)
        if has_qk_bias:
            bq_row = cpool.tile([1, C], f32)
            bk_row = cpool.tile([1, C], f32)
        sel_sb = cpool.tile([128, 8], f32)
        selT_sb = cpool.tile([8, 128], f32)
        ones1 = cpool.tile([1, 128], bf16)
        maskh = cpool.tile([128, 128], bf16)
        nc.sync.dma_start(gammaP[:], gammaP_d)
        nc.sync.dma_start(betaP[:], betaP_d)
        nc.sync.dma_start(bvP[:], bvP_d)
        nc.sync.dma_start(bpP[:], bpP_d)
        if has_qk_bias:
            nc.sync.dma_start(bq_row[:], bq_row_d)
            nc.sync.dma_start(bk_row[:], bk_row_d)
        nc.sync.dma_start(sel_sb[:], sel_d)
        nc.sync.dma_start(selT_sb[:], selT_d)
        nc.sync.dma_start(ones1[:], ones1_d)
        nc.sync.dma_start(maskh[:], maskh_d)
        eps_t = cpool.tile([8, 1], f32)
        nc.vector.memset(eps_t[:], EPS)

        bn_tiles = {}
        x_tiles = {}
        ab_tiles = {}

        def x_macro_ap(b, i):
            return x_d[b, :, TT * i:TT * i + TT].rearrange(
                "(j p) t -> p j t", p=128)

        def emit_pass0_macro(b, i):
            if b not in bn_tiles:
                bn_tiles[b] = stpool.tile([128, 4, NMi * 12], f32,
                                          tag="bnall", name=f"bnall{b}")
                x_tiles[b] = xcache.tile([128, 4, T], bf16, tag="xc",
                                         name=f"xc{b}")
            bnall = bn_tiles[b]
            xbf = x_tiles[b]
            HT = TT // 2
            for h in range(2):
                ih = 2 * i + h
                xt = xin.tile([128, 4, HT], f32, tag="xt",
                              name=f"x0_{b}_{ih}")
                nc.sync.dma_start(
                    xt[:], x_d[b, :, HT * ih:HT * ih + HT].rearrange(
                        "(j p) t -> p j t", p=128))
                nc.gpsimd.tensor_copy(xbf[:, :, HT * ih:HT * ih + HT], xt[:])
                for j in range(4):
                    nc.vector.bn_stats(bnall[:, j, 6 * ih:6 * ih + 6],
                                       xt[:, j, :])

        def emit_finalize(b):
            # GN stats aggregation (identical machinery to the proven
            # baseline): bn_aggr per chunk -> per-channel mean/meansq,
            # selector matmuls aggregate 16-channel groups -> a, b.
            bnall = bn_tiles[b]
            statsc = stpool.tile([128, 4, 2], f32, tag="statsc",
                                 name=f"statsc{b}")
            stats2 = stpool.tile([128, 8], f32, tag="stats2",
                                 name=f"stats2_{b}")
            for j in range(4):
                nc.vector.bn_aggr(statsc[:, j, :], bnall[:, j, :])
                nc.vector.tensor_copy(stats2[:, 2 * j:2 * j + 1],
                                      statsc[:, j, 0:1])
                nc.vector.scalar_tensor_tensor(
                    stats2[:, 2 * j + 1:2 * j + 2],
                    in0=statsc[:, j, 0:1], scalar=statsc[:, j, 0:1],
                    in1=statsc[:, j, 1:2], op0=ALU.mult, op1=ALU.add)
            aT = stpool.tile([128, 4], f32, tag="aT", name=f"aT{b}")
            bvec = stpool.tile([128, 4], f32, tag="bvec", name=f"bvec{b}")
            with tc.tile_pool(name=f"st_ps{b}", bufs=2, space="PSUM") as stps:
                gsum_ps = stps.tile([8, 8], f32, name=f"gsum{b}")
                nc.tensor.matmul(gsum_ps[:], sel_sb[:], stats2[:])
                gs = stpool.tile([8, 4, 2], f32, tag="gs", name=f"gs{b}")
                nc.vector.tensor_scalar_mul(gs[:], gsum_ps.rearrange(
                    "p (j s) -> p j s", s=2), 1.0 / CG)
                mg2 = stpool.tile([8, 4], f32, tag="mg2", name=f"mg2_{b}")
                nc.vector.tensor_mul(mg2[:], gs[:, :, 0], gs[:, :, 0])
                gvar = stpool.tile([8, 4], f32, tag="gvar", name=f"gvar{b}")
                nc.vector.tensor_sub(gvar[:], gs[:, :, 1], mg2[:])
                gstd = stpool.tile([8, 4], f32, tag="gstd", name=f"gstd{b}")
                nc.scalar.activation(gstd[:], gvar[:], AF.Sqrt, bias=eps_t[:])
                ginv = stpool.tile([8, 4], f32, tag="ginv", name=f"ginv{b}")
                nc.vector.reciprocal(ginv[:], gstd[:])
                gb = stpool.tile([8, 4, 2], f32, tag="gb", name=f"gb{b}")
                nc.vector.tensor_copy(gb[:, :, 0], gs[:, :, 0])
                nc.vector.tensor_copy(gb[:, :, 1], ginv[:])
                chB_ps = stps.tile([128, 8], f32, name=f"chB{b}")
                nc.tensor.matmul(chB_ps[:], selT_sb[:], gb.rearrange(
                    "p j s -> p (j s)"))
                chB = chB_ps.rearrange("p (j s) -> p j s", s=2)
                nc.vector.tensor_mul(aT[:], gammaP[:], chB[:, :, 1])
                tmpb = stpool.tile([128, 4], f32, tag="tmpb", name=f"tmpb{b}")
                nc.vector.tensor_mul(tmpb[:], chB[:, :, 0], aT[:])
                nc.vector.tensor_sub(bvec[:], betaP[:], tmpb[:])
            # fold GN affine into q/k weights: wqf = a * wqt (bf16)
            wqf = wfpool.tile([128, 4, C], bf16, tag="wqf", name=f"wqf{b}")
            wkf = wfpool.tile([128, 4, C], bf16, tag="wkf", name=f"wkf{b}")
            for j in range(4):
                nc.vector.tensor_scalar_mul(wqf[:, j, :], wqt_sb[:, j, :],
                                            aT[:, j:j + 1])
                nc.vector.tensor_scalar_mul(wkf[:, j, :], wkt_sb[:, j, :],
                                            aT[:, j:j + 1])
            bvec_bf = stpool.tile([128, 4], bf16, tag="bvbf", name=f"bvbf{b}")
            nc.vector.tensor_copy(bvec_bf[:], bvec[:])
            # cq = Wq b (+bq), ck = Wk b (+bk): [1, C] rows
            cqf = cqpool.tile([1, C], bf16, tag="cqf", name=f"cqf{b}")
            ckf = cqpool.tile([1, C], bf16, tag="ckf", name=f"ckf{b}")
            with tc.tile_pool(name=f"cq_ps{b}", bufs=2, space="PSUM") as cqps:
                cq_ps = cqps.tile([1, C], f32, name=f"cqp{b}")
                ck_ps = cqps.tile([1, C], f32, name=f"ckp{b}")
                for j in range(4):
                    nc.tensor.matmul(cq_ps[:], bvec_bf[:, j:j + 1],
                                     wqt_sb[:, j, :],
                                     start=(j == 0), stop=(j == 3))
                    nc.tensor.matmul(ck_ps[:], bvec_bf[:, j:j + 1],
                                     wkt_sb[:, j, :],
                                     start=(j == 0), stop=(j == 3))
                if has_qk_bias:
                    nc.vector.tensor_add(cqf[:], cq_ps[:], bq_row[:])
                    nc.vector.tensor_add(ckf[:], ck_ps[:], bk_row[:])
                else:
                    nc.vector.tensor_copy(cqf[:], cq_ps[:])
                    nc.vector.tensor_copy(ckf[:], ck_ps[:])
            ab_tiles[b] = (aT, bvec, bvec_bf, wqf, wkf, cqf, ckf)

        def emit_passA(b, interleave_next):
            aT, bvec, bvec_bf, wqf, wkf, cqf, ckf = ab_tiles[b]
            xbf = x_tiles[b]
            with ExitStack() as est_a:
                qps_pool = est_a.enter_context(
                    tc.tile_pool(name=f"q_ps{b}", bufs=3, space="PSUM"))
                kps_pool = est_a.enter_context(
                    tc.tile_pool(name=f"k_ps{b}", bufs=3, space="PSUM"))
                wps_pool = est_a.enter_context(
                    tc.tile_pool(name=f"w_ps{b}", bufs=1, space="PSUM"))
                w_ps = wps_pool.tile([128, 4, 128], f32, name=f"wps{b}")
                eqkp = {}

                def emit_w_macro(i):
                    # delayed one macro so the PE queue never stalls on
                    # the scalar/vector softmax chain
                    eq_i, kp_i = eqkp.pop(i)
                    for s in range(4):
                        first = (i == 0 and s == 0)
                        last = (i == NMi - 1 and s == 3)
                        for m in range(4):
                            nc.tensor.matmul(
                                w_ps[:, m, :],
                                kp_i[:, s, 128 * m:128 * m + 128],
                                eq_i[:, s, 128 * m:128 * m + 128],
                                start=(first and m == 0),
                                stop=(last and m == 3),
                                skip_group_check=True)

                for i in range(NMi):
                    eq = eqpool.tile([128, 4, TT], bf16, tag="eq",
                                     name=f"eq_{b}_{i}")
                    ek = ekpool.tile([128, 4, TT], bf16, tag="ek",
                                     name=f"ek_{b}_{i}")
                    for s in range(4):
                        qps = qps_pool.tile([128, TT], f32, tag="q",
                                            name=f"qps_{b}_{i}_{s}")
                        kps = kps_pool.tile([128, TT], f32, tag="k",
                                            name=f"kps_{b}_{i}_{s}")
                        for j in range(4):
                            lhs = xbf[:, j, TT * i + 128 * s:
                                      TT * i + 128 * s + 128]
                            nc.tensor.matmul(qps[:], lhs, wqf[:, j, :],
                                             start=(j == 0), stop=False)
                            nc.tensor.matmul(kps[:], lhs, wkf[:, j, :],
                                             start=(j == 0), stop=False)
                        nc.tensor.matmul(qps[:], ones1[:], cqf[:],
                                         start=False, stop=True)
                        nc.tensor.matmul(kps[:], ones1[:], ckf[:],
                                         start=False, stop=True)
                        nc.scalar.activation(eq[:, s, :], qps[:], AF.Exp)
                        nc.scalar.activation(ek[:, s, :], kps[:], AF.Exp)
                    sq = smpool.tile([128, 4 * NH], bf16, tag="sq",
                                     name=f"sq_{b}_{i}")
                    sk = smpool.tile([128, 4 * NH], bf16, tag="sk",
                                     name=f"sk_{b}_{i}")
                    with nc.allow_low_precision(reason="softmax denom bf16"):
                        nc.vector.tensor_reduce(
                            sq[:], eq.rearrange("p s (n c) -> p s n c", c=HC),
                            axis=AX.X, op=ALU.add)
                        nc.vector.tensor_reduce(
                            sk[:], ek.rearrange("p s (n c) -> p s n c", c=HC),
                            axis=AX.X, op=ALU.add)
                    ss = smpool.tile([128, 4 * NH], bf16, tag="ss",
                                     name=f"ss_{b}_{i}")
                    nc.vector.tensor_mul(ss[:], sq[:], sk[:])
                    rr = smpool.tile([128, 4 * NH], bf16, tag="rr",
                                     name=f"rr_{b}_{i}")
                    with nc.allow_low_precision(reason="softmax denom bf16"):
                        nc.vector.reciprocal(rr[:], ss[:])
                    kp = kppool.tile([128, 4, TT], bf16, tag="kp",
                                     name=f"kp_{b}_{i}")
                    nc.vector.tensor_mul(
                        kp.rearrange("p s (n c) -> p s n c", c=HC),
                        ek.rearrange("p s (n c) -> p s n c", c=HC),
                        rr.rearrange("p (s n) -> p s n", s=4)[
                            :, :, :, None].broadcast_to([128, 4, NH, HC]))
                    eqkp[i] = (eq, kp)
                    if i > 0:
                        emit_w_macro(i - 1)
                    if interleave_next is not None:
                        emit_pass0_macro(interleave_next, i)
                emit_w_macro(NMi - 1)
                # mask cross-head blocks: wm[d_loc, m, c_loc] (bf16)
                wm = wsbpool.tile([128, 4, 128], bf16, tag="wm",
                                  name=f"wm{b}")
                nc.vector.tensor_mul(
                    wm[:], w_ps[:],
                    maskh[:, None, :].broadcast_to([128, 4, 128]))
            return wm

        def emit_mfuse(b, wm):
            # M = Wp * BD(w) * Wv ; mt = M''^T = diag(a) M^T (bf16, [i, o])
            # cP = M b (+ Wp BD(w) bv) + bp
            aT, bvec, bvec_bf, wqf, wkf, cqf, ckf = ab_tiles[b]
            a_sb = wfpool.tile([128, 4, C], bf16, tag="wqf", name=f"a_sb{b}")
            mt_sb = mpool.tile([128, 4, C], bf16, tag="mt", name=f"mt{b}")
            cPfull = stpool.tile([128, 4], f32, tag="cPf", name=f"cPf{b}")
            with tc.tile_pool(name=f"m_ps{b}", bufs=2, space="PSUM") as mps, \
                 tc.tile_pool(name=f"c_ps{b}", bufs=2, space="PSUM") as cps:
                # A[c, i] = sum_d w[c, d] Wv[d, i]   (per 128-block m)
                for m in range(4):
                    a_ps = mps.tile([128, C], f32, tag="aps", name=f"aps{b}_{m}")
                    nc.tensor.matmul(a_ps[:], wm[:, m, :], wv_sb[:, m, :])
                    nc.scalar.activation(a_sb[:, m, :], a_ps[:], AF.Identity)
                # MT[i, o] = sum_c A[c, i] wpt[c, o]  -> M''T = a_i * MT
                for ni in range(4):
                    mt_ps = mps.tile([128, C], f32, tag="mtps", name=f"mtps{b}_{ni}")
                    for m in range(4):
                        nc.tensor.matmul(
                            mt_ps[:], a_sb[:, m, 128 * ni:128 * ni + 128],
                            wpt_sb[:, m, :],
                            start=(m == 0), stop=(m == 3))
                    nc.vector.tensor_scalar_mul(mt_sb[:, ni, :], mt_ps[:],
                                                aT[:, ni:ni + 1])
                # cP = M b: sum_i (a_i M[o,i]) * (b_i / a_i)
                rA = stpool.tile([128, 4], f32, tag="rA", name=f"rA{b}")
                nc.vector.reciprocal(rA[:], aT[:])
                bova = stpool.tile([128, 4], bf16, tag="bova", name=f"bova{b}")
                nc.vector.tensor_mul(bova[:], bvec[:], rA[:])
                cbv_sb = None
                if has_bv:
                    bvP_bf = stpool.tile([128, 4], bf16, tag="bvPbf",
                                         name=f"bvPbf{b}")
                    nc.vector.tensor_copy(bvP_bf[:], bvP[:])
                    cbv_ps = cps.tile([128, 4], f32, tag="cbvps", name=f"cbvps{b}")
                    for m in range(4):
                        nc.tensor.matmul(cbv_ps[:, m:m + 1], wm[:, m, :],
                                         bvP_bf[:, m:m + 1],
                                         start=(m == 0), stop=(m == 3),
                                         skip_group_check=True)
                    cbv_sb = stpool.tile([128, 4], bf16, tag="cbv",
                                         name=f"cbv{b}")
                    nc.vector.tensor_copy(cbv_sb[:], cbv_ps[:])
                cP_ps = cps.tile([128, 4], f32, tag="cPps", name=f"cPps{b}")
                n_mm = 4 * (4 + (4 if has_bv else 0))
                idx = 0
                for no in range(4):
                    for ji in range(4):
                        nc.tensor.matmul(
                            cP_ps[:, no:no + 1],
                            mt_sb[:, ji, 128 * no:128 * no + 128],
                            bova[:, ji:ji + 1],
                            start=(idx == 0), stop=(idx == n_mm - 1),
                            skip_group_check=True)
                        idx += 1
                    if has_bv:
                        for m in range(4):
                            nc.tensor.matmul(
                                cP_ps[:, no:no + 1],
                                wpt_sb[:, m, 128 * no:128 * no + 128],
                                cbv_sb[:, m:m + 1],
                                start=(idx == 0), stop=(idx == n_mm - 1),
                                skip_group_check=True)
                            idx += 1
                nc.vector.tensor_add(cPfull[:], cP_ps[:], bpP[:])
            return mt_sb, cPfull

        def emit_passB(b, mt_sb, cPfull):
            xbf = x_tiles[b]
            with ExitStack() as est_b:
                pj_pool = est_b.enter_context(
                    tc.tile_pool(name=f"pj_ps{b}", bufs=3, space="PSUM"))
                for i in range(NMi):
                    ot = opool.tile([128, 4, TT], bf16, tag="ot",
                                    name=f"ot_{b}_{i}")
                    for no in range(4):
                        pj = pj_pool.tile([128, TT], f32, tag="pj",
                                          name=f"pj_{b}_{i}_{no}")
                        for ji in range(4):
                            nc.tensor.matmul(
                                pj[:], mt_sb[:, ji, 128 * no:128 * no + 128],
                                xbf[:, ji, TT * i:TT * i + TT],
                                start=(ji == 0), stop=(ji == 3))
                        nc.vector.scalar_tensor_tensor(
                            ot[:, no, :], in0=pj[:],
                            scalar=cPfull[:, no:no + 1],
                            in1=xbf[:, no, TT * i:TT * i + TT],
                            op0=ALU.add, op1=ALU.add)
                    nc.sync.dma_start(
                        out_d[b, :, TT * i:TT * i + TT].rearrange(
                            "(j p) t -> p j t", p=128),
                        ot[:])

        # schedule: pass0(0); per batch: finalize, passA (next batch's
        # pass0 interleaved), M-fuse, passB.
        for i in range(NMi):
            emit_pass0_macro(0, i)
        for b in range(B):
            emit_finalize(b)
            wm = emit_passA(b, b + 1 if b + 1 < B else None)
            mt_sb, cPfull = emit_mfuse(b, wm)
            emit_passB(b, mt_sb, cPfull)

    nc.compile()
    return nc


def _to_part4(vec):
    # [512] -> [128, 4]: column j = channels 128j..128j+127
    return np.ascontiguousarray(vec.reshape(4, 128).T)


def _host_prep(x, gn_scale, gn_bias, wq, bq, wk, bk, wv, bv, wp, bp):
    import ml_dtypes
    bf = ml_dtypes.bfloat16
    sel = np.zeros((128, 8), dtype=np.float32)
    for p in range(128):
        sel[p, p // CG] = 1.0
    maskh = np.zeros((128, 128), dtype=np.float32)
    for p in range(128):
        maskh[p, (p // HC) * HC:(p // HC) * HC + HC] = 1.0
    consts = {
        "wqt": np.ascontiguousarray(wq.T).astype(bf),
        "wkt": np.ascontiguousarray(wk.T).astype(bf),
        "wv": np.ascontiguousarray(wv).astype(bf),
        "wpt": np.ascontiguousarray(wp.T).astype(bf),
        "gammaP": _to_part4(np.asarray(gn_scale)).astype(np.float32),
        "betaP": _to_part4(np.asarray(gn_bias)).astype(np.float32),
        "bq_row": np.asarray(bq).reshape(1, C).astype(np.float32),
        "bk_row": np.asarray(bk).reshape(1, C).astype(np.float32),
        "bvP": _to_part4(np.asarray(bv)).astype(np.float32),
        "bpP": _to_part4(np.asarray(bp)).astype(np.float32),
        "sel": sel,
        "selT": np.ascontiguousarray(sel.T),
        "ones1": np.ones((1, 128), dtype=np.float32).astype(bf),
        "maskh": maskh.astype(bf),
    }
    return consts


_NC_CACHE = {}


def kernel(x, gn_scale, gn_bias, wq, bq, wk, bk, wv, bv, wp, bp):
    from concourse.bass_utils import run_bass_kernel_spmd

    x = np.asarray(x, dtype=np.float32)
    consts = _host_prep(x, gn_scale, gn_bias, wq, bq, wk, bk, wv, bv, wp, bp)

    has_qk_bias = bool(np.any(np.asarray(bq)) or np.any(np.asarray(bk)))
    has_bv = bool(np.any(np.asarray(bv)))
    key = (B_SHARD, T_FULL, has_qk_bias, has_bv)
    if key not in _NC_CACHE:
        _NC_CACHE[key] = build_nc(B_SHARD, T_FULL, has_qk_bias=has_qk_bias,
                                  has_bv=has_bv)
    nc = _NC_CACHE[key]

    in_maps = []
    for c in range(N_CORES):
        m = dict(consts)
        m["x"] = np.ascontiguousarray(x[B_SHARD * c:B_SHARD * (c + 1)])
        in_maps.append(m)
    res = run_bass_kernel_spmd(nc, in_maps, core_ids=list(range(N_CORES)))
    out = np.concatenate([np.asarray(r["out"]).astype(np.float32)
                          for r in res.results], axis=0)
    return out


# revision 16
# speedup vs baseline: 1.0622x; 1.0622x over previous
"""Trainium2 Bass kernel for nn_ChannelAttnBlock (GroupNorm + channel attention).

Self-contained: takes FULL unsharded inputs, shards batch over 8 NeuronCores
(2 batches/core), runs one SPMD NEFF, gathers the full output.

Per-core dataflow (B=2 batches, C=512 channels, T=8192), v2:
  pass 0: stream x (f32, single HBM read), cast to bf16 SBUF cache (gpsimd),
          bn_stats -> per-channel mean/var; selector matmuls aggregate the
          32 GN groups -> per-channel affine a, b. GN affine is folded into
          the q/k weights (wqf = a*wqt, bf16) and bias rows cq = Wq b + bq.
  pass A: qT/kT = x^T @ wqf + cq (t on partitions, bf16 matmuls; channel
          softmax is a free-dim segment reduce); exp on ACT (bf16 out);
          softmax denominators folded into kp = ek/(Sq*Sk) (bf16);
          w accumulated in PSUM as per-128-block outer products over t.
  M-fuse: h2 = BD(w) v and out-proj collapse into a single matrix:
          out = x + M''^T(ish) where M = Wp*BD(w)*Wv, M'' = M*diag(a),
          cP = M b (+ Wp BD(w) bv + bp). Computed on-chip per batch with
          a handful of small matmuls (no transposes needed: MT computed
          directly as A^T-free chain).
  pass B: out = x + M''x + cP: 16 bf16 matmuls per macro from the bf16
          x cache; residual+bias via gpsimd STT; bf16 out written to HBM.
"""

import numpy as np

C = 512
NH = 16      # heads
HC = 32      # channels/head
G = 32       # groupnorm groups
CG = C // G  # 16 channels per group
EPS = 1e-6

N_CORES = 8
B_FULL = 16
T_FULL = 8192
B_SHARD = B_FULL // N_CORES  # 2
TT = 512                     # t macro-tile
NM = T_FULL // TT            # 16 macros per batch


def build_nc(B, T, has_qk_bias=True, has_bv=True, debug=False):
    import concourse.tile as tile
    import concourse.mybir as mybir
    from concourse import bacc

    NMi = T // TT
    f32 = mybir.dt.float32
    bf16 = mybir.dt.bfloat16
    AF = mybir.ActivationFunctionType
    ALU = mybir.AluOpType
    AX = mybir.AxisListType

    nc = bacc.Bacc("TRN2", target_bir_lowering=False, debug=debug)

    x_d = nc.dram_tensor("x", [B, C, T], f32, kind="ExternalInput").ap()
    wqt_d = nc.dram_tensor("wqt", [C, C], bf16, kind="ExternalInput").ap()
    wkt_d = nc.dram_tensor("wkt", [C, C], bf16, kind="ExternalInput").ap()
    wv_d = nc.dram_tensor("wv", [C, C], bf16, kind="ExternalInput").ap()
    wpt_d = nc.dram_tensor("wpt", [C, C], bf16, kind="ExternalInput").ap()
    gammaP_d = nc.dram_tensor("gammaP", [128, 4], f32, kind="ExternalInput").ap()
    betaP_d = nc.dram_tensor("betaP", [128, 4], f32, kind="ExternalInput").ap()
    if has_qk_bias:
        bq_row_d = nc.dram_tensor("bq_row", [1, C], f32,
                                  kind="ExternalInput").ap()
        bk_row_d = nc.dram_tensor("bk_row", [1, C], f32,
                                  kind="ExternalInput").ap()
    bvP_d = nc.dram_tensor("bvP", [128, 4], f32, kind="ExternalInput").ap()
    bpP_d = nc.dram_tensor("bpP", [128, 4], f32, kind="ExternalInput").ap()
    sel_d = nc.dram_tensor("sel", [128, 8], f32, kind="ExternalInput").ap()
    selT_d = nc.dram_tensor("selT", [8, 128], f32, kind="ExternalInput").ap()
    ones1_d = nc.dram_tensor("ones1", [1, 128], bf16, kind="ExternalInput").ap()
    maskh_d = nc.dram_tensor("maskh", [128, 128], bf16, kind="ExternalInput").ap()
    out_d = nc.dram_tensor("out", [B, C, T], bf16, kind="ExternalOutput").ap()

    from contextlib import ExitStack

    with tile.TileContext(nc) as tc, ExitStack() as est:
        p = lambda name, bufs: est.enter_context(
            tc.tile_pool(name=name, bufs=bufs))
        wpool = p("wpool", 1)
        cpool = p("cpool", 1)
        xcache = p("xcache", 2)
        xin = p("xin", 2)
        stpool = p("stpool", 2)
        wfpool = p("wfpool", 1)
        cqpool = p("cqpool", 1)
        eqpool = p("eqpool", 2)
        ekpool = p("ekpool", 1)
        kppool = p("kppool", 2)
        smpool = p("smpool", 2)
        wsbpool = p("wsbpool", 1)
        mpool = p("mpool", 1)
        opool = p("opool", 2)

        # ---- load weights & constants ----
        wqt_sb = wpool.tile([128, 4, C], bf16)
        wkt_sb = wpool.tile([128, 4, C], bf16)
        wv_sb = wpool.tile([128, 4, C], bf16)
        wpt_sb = wpool.tile([128, 4, C], bf16)
        for j in range(4):
            nc.sync.dma_start(wqt_sb[:, j, :], wqt_d[128 * j:128 * j + 128, :])
            nc.sync.dma_start(wkt_sb[:, j, :], wkt_d[128 * j:128 * j + 128, :])
            nc.sync.dma_start(wv_sb[:, j, :], wv_d[128 * j:128 * j + 128, :])
            nc.sync.dma_start(wpt_sb[:, j, :], wpt_d[128 * j:128 * j + 128, :])
        gammaP = cpool.tile([128, 4], f32)
        betaP = cpool.tile([128, 4], f32)
        bvP = cpool.tile([128, 4], f32)
        bpP = cpool.tile([128, 4], f32)
        if has_qk_bias:
            bq_row = cpool.tile([1, C], f32)
            bk_row = cpool.tile([1, C], f32)
        sel_sb = cpool.tile([128, 8], f32)
        selT_sb = cpool.tile([8, 128], f32)
        ones1 = cpool.tile([1, 128], bf16)
        maskh = cpool.tile([128, 128], bf16)
        nc.sync.dma_start(gammaP[:], gammaP_d)
        nc.sync.dma_start(betaP[:], betaP_d)
        nc.sync.dma_start(bvP[:], bvP_d)
        nc.sync.dma_start(bpP[:], bpP_d)
        if has_qk_bias:
            nc.sync.dma_start(bq_row[:], bq_row_d)
            nc.sync.dma_start(bk_row[:], bk_row_d)
        nc.sync.dma_start(sel_sb[:], sel_d)
        nc.sync.dma_start(selT_sb[:], selT_d)
        nc.sync.dma_start(ones1[:], ones1_d)
        nc.sync.dma_start(maskh[:], maskh_d)
        eps_t = cpool.tile([8, 1], f32)
        nc.vector.memset(eps_t[:], EPS)

        bn_tiles = {}
        x_tiles = {}
        ab_tiles = {}

        def x_macro_ap(b, i):
            return x_d[b, :, TT * i:TT * i + TT].rearrange(
                "(j p) t -> p j t", p=128)

        def emit_pass0_macro(b, i):
            if b not in bn_tiles:
                bn_tiles[b] = stpool.tile([128, 4, NMi * 12], f32,
                                          tag="bnall", name=f"bnall{b}")
                x_tiles[b] = xcache.tile([128, 4, T], bf16, tag="xc",
                                         name=f"xc{b}")
            bnall = bn_tiles[b]
            xbf = x_tiles[b]
            HT = TT // 2
            for h in range(2):
                ih = 2 * i + h
                xt = xin.tile([128, 4, HT], f32, tag="xt",
                              name=f"x0_{b}_{ih}")
                nc.sync.dma_start(
                    xt[:], x_d[b, :, HT * ih:HT * ih + HT].rearrange(
                        "(j p) t -> p j t", p=128))
                # batch 0's cast rides the idle Scalar engine so the
                # pass-0 head is DMA-bound, not gpsimd-bound; batch 1's
                # cast overlaps passA(b0) on the otherwise-idle GpSimd.
                if b == 0:
                    nc.scalar.copy(xbf[:, :, HT * ih:HT * ih + HT], xt[:])
                else:
                    nc.gpsimd.tensor_copy(xbf[:, :, HT * ih:HT * ih + HT],
                                          xt[:])
                for j in range(4):
                    nc.vector.bn_stats(bnall[:, j, 6 * ih:6 * ih + 6],
                                       xt[:, j, :])

        def emit_finalize(b):
            # GN stats aggregation (identical machinery to the proven
            # baseline): bn_aggr per chunk -> per-channel mean/meansq,
            # selector matmuls aggregate 16-channel groups -> a, b.
            bnall = bn_tiles[b]
            statsc = stpool.tile([128, 4, 2], f32, tag="statsc",
                                 name=f"statsc{b}")
            stats2 = stpool.tile([128, 8], f32, tag="stats2",
                                 name=f"stats2_{b}")
            for j in range(4):
                nc.vector.bn_aggr(statsc[:, j, :], bnall[:, j, :])
                nc.vector.tensor_copy(stats2[:, 2 * j:2 * j + 1],
                                      statsc[:, j, 0:1])
                nc.vector.scalar_tensor_tensor(
                    stats2[:, 2 * j + 1:2 * j + 2],
                    in0=statsc[:, j, 0:1], scalar=statsc[:, j, 0:1],
                    in1=statsc[:, j, 1:2], op0=ALU.mult, op1=ALU.add)
            aT = stpool.tile([128, 4], f32, tag="aT", name=f"aT{b}")
            bvec = stpool.tile([128, 4], f32, tag="bvec", name=f"bvec{b}")
            with tc.tile_pool(name=f"st_ps{b}", bufs=2, space="PSUM") as stps:
                gsum_ps = stps.tile([8, 8], f32, name=f"gsum{b}")
                nc.tensor.matmul(gsum_ps[:], sel_sb[:], stats2[:])
                gs = stpool.tile([8, 4, 2], f32, tag="gs", name=f"gs{b}")
                nc.vector.tensor_scalar_mul(gs[:], gsum_ps.rearrange(
                    "p (j s) -> p j s", s=2), 1.0 / CG)
                mg2 = stpool.tile([8, 4], f32, tag="mg2", name=f"mg2_{b}")
                nc.vector.tensor_mul(mg2[:], gs[:, :, 0], gs[:, :, 0])
                gvar = stpool.tile([8, 4], f32, tag="gvar", name=f"gvar{b}")
                nc.vector.tensor_sub(gvar[:], gs[:, :, 1], mg2[:])
                gstd = stpool.tile([8, 4], f32, tag="gstd", name=f"gstd{b}")
                nc.scalar.activation(gstd[:], gvar[:], AF.Sqrt, bias=eps_t[:])
                ginv = stpool.tile([8, 4], f32, tag="ginv", name=f"ginv{b}")
                nc.vector.reciprocal(ginv[:], gstd[:])
                gb = stpool.tile([8, 4, 2], f32, tag="gb", name=f"gb{b}")
                nc.vector.tensor_copy(gb[:, :, 0], gs[:, :, 0])
                nc.vector.tensor_copy(gb[:, :, 1], ginv[:])
                chB_ps = stps.tile([128, 8], f32, name=f"chB{b}")
                nc.tensor.matmul(chB_ps[:], selT_sb[:], gb.rearrange(
                    "p j s -> p (j s)"))
                chB = chB_ps.rearrange("p (j s) -> p j s", s=2)
                nc.vector.tensor_mul(aT[:], gammaP[:], chB[:, :, 1])
                tmpb = stpool.tile([128, 4], f32, tag="tmpb", name=f"tmpb{b}")
                nc.vector.tensor_mul(tmpb[:], chB[:, :, 0], aT[:])
                nc.vector.tensor_sub(bvec[:], betaP[:], tmpb[:])
            # fold GN affine into q/k weights: wqf = a * wqt (bf16)
            wqf = wfpool.tile([128, 4, C], bf16, tag="wqf", name=f"wqf{b}")
            wkf = wfpool.tile([128, 4, C], bf16, tag="wkf", name=f"wkf{b}")
            for j in range(4):
                nc.vector.tensor_scalar_mul(wqf[:, j, :], wqt_sb[:, j, :],
                                            aT[:, j:j + 1])
                nc.vector.tensor_scalar_mul(wkf[:, j, :], wkt_sb[:, j, :],
                                            aT[:, j:j + 1])
            bvec_bf = stpool.tile([128, 4], bf16, tag="bvbf", name=f"bvbf{b}")
            nc.vector.tensor_copy(bvec_bf[:], bvec[:])
            # cq = Wq b (+bq), ck = Wk b (+bk): [1, C] rows
            cqf = cqpool.tile([1, C], bf16, tag="cqf", name=f"cqf{b}")
            ckf = cqpool.tile([1, C], bf16, tag="ckf", name=f"ckf{b}")
            with tc.tile_pool(name=f"cq_ps{b}", bufs=2, space="PSUM") as cqps:
                cq_ps = cqps.tile([1, C], f32, name=f"cqp{b}")
                ck_ps = cqps.tile([1, C], f32, name=f"ckp{b}")
                for j in range(4):
                    nc.tensor.matmul(cq_ps[:], bvec_bf[:, j:j + 1],
                                     wqt_sb[:, j, :],
                                     start=(j == 0), stop=(j == 3))
                    nc.tensor.matmul(ck_ps[:], bvec_bf[:, j:j + 1],
                                     wkt_sb[:, j, :],
                                     start=(j == 0), stop=(j == 3))
                if has_qk_bias:
                    nc.vector.tensor_add(cqf[:], cq_ps[:], bq_row[:])
                    nc.vector.tensor_add(ckf[:], ck_ps[:], bk_row[:])
                else:
                    nc.vector.tensor_copy(cqf[:], cq_ps[:])
                    nc.vector.tensor_copy(ckf[:], ck_ps[:])
            ab_tiles[b] = (aT, bvec, bvec_bf, wqf, wkf, cqf, ckf)

        def emit_passA(b, interleave_next):
            aT, bvec, bvec_bf, wqf, wkf, cqf, ckf = ab_tiles[b]
            xbf = x_tiles[b]
            with ExitStack() as est_a:
                qps_pool = est_a.enter_context(
                    tc.tile_pool(name=f"q_ps{b}", bufs=3, space="PSUM"))
                kps_pool = est_a.enter_context(
                    tc.tile_pool(name=f"k_ps{b}", bufs=3, space="PSUM"))
                wps_pool = est_a.enter_context(
                    tc.tile_pool(name=f"w_ps{b}", bufs=1, space="PSUM"))
                w_ps = wps_pool.tile([128, 4, 128], f32, name=f"wps{b}")
                eqkp = {}

                def emit_w_macro(i):
                    # delayed one macro so the PE queue never stalls on
                    # the scalar/vector softmax chain
                    eq_i, kp_i = eqkp.pop(i)
                    for s in range(4):
                        first = (i == 0 and s == 0)
                        last = (i == NMi - 1 and s == 3)
                        for m in range(4):
                            nc.tensor.matmul(
                                w_ps[:, m, :],
                                kp_i[:, s, 128 * m:128 * m + 128],
                                eq_i[:, s, 128 * m:128 * m + 128],
                                start=(first and m == 0),
                                stop=(last and m == 3),
                                skip_group_check=True)

                for i in range(NMi):
                    eq = eqpool.tile([128, 4, TT], bf16, tag="eq",
                                     name=f"eq_{b}_{i}")
                    ek = ekpool.tile([128, 4, TT], bf16, tag="ek",
                                     name=f"ek_{b}_{i}")
                    for s in range(4):
                        qps = qps_pool.tile([128, TT], f32, tag="q",
                                            name=f"qps_{b}_{i}_{s}")
                        kps = kps_pool.tile([128, TT], f32, tag="k",
                                            name=f"kps_{b}_{i}_{s}")
                        for j in range(4):
                            lhs = xbf[:, j, TT * i + 128 * s:
                                      TT * i + 128 * s + 128]
                            nc.tensor.matmul(qps[:], lhs, wqf[:, j, :],
                                             start=(j == 0), stop=False)
                            nc.tensor.matmul(kps[:], lhs, wkf[:, j, :],
                                             start=(j == 0), stop=False)
                        nc.tensor.matmul(qps[:], ones1[:], cqf[:],
                                         start=False, stop=True)
                        nc.tensor.matmul(kps[:], ones1[:], ckf[:],
                                         start=False, stop=True)
                        nc.scalar.activation(eq[:, s, :], qps[:], AF.Exp)
                        nc.scalar.activation(ek[:, s, :], kps[:], AF.Exp)
                    sq = smpool.tile([128, 4 * NH], bf16, tag="sq",
                                     name=f"sq_{b}_{i}")
                    sk = smpool.tile([128, 4 * NH], bf16, tag="sk",
                                     name=f"sk_{b}_{i}")
                    with nc.allow_low_precision(reason="softmax denom bf16"):
                        nc.vector.tensor_reduce(
                            sq[:], eq.rearrange("p s (n c) -> p s n c", c=HC),
                            axis=AX.X, op=ALU.add)
                        nc.vector.tensor_reduce(
                            sk[:], ek.rearrange("p s (n c) -> p s n c", c=HC),
                            axis=AX.X, op=ALU.add)
                    ss = smpool.tile([128, 4 * NH], bf16, tag="ss",
                                     name=f"ss_{b}_{i}")
                    nc.vector.tensor_mul(ss[:], sq[:], sk[:])
                    rr = smpool.tile([128, 4 * NH], bf16, tag="rr",
                                     name=f"rr_{b}_{i}")
                    with nc.allow_low_precision(reason="softmax denom bf16"):
                        nc.vector.reciprocal(rr[:], ss[:])
                    kp = kppool.tile([128, 4, TT], bf16, tag="kp",
                                     name=f"kp_{b}_{i}")
                    nc.vector.tensor_mul(
                        kp.rearrange("p s (n c) -> p s n c", c=HC),
                        ek.rearrange("p s (n c) -> p s n c", c=HC),
                        rr.rearrange("p (s n) -> p s n", s=4)[
                            :, :, :, None].broadcast_to([128, 4, NH, HC]))
                    eqkp[i] = (eq, kp)
                    if i > 0:
                        emit_w_macro(i - 1)
                    if interleave_next is not None:
                        emit_pass0_macro(interleave_next, i)
                emit_w_macro(NMi - 1)
                # mask cross-head blocks: wm[d_loc, m, c_loc] (bf16)
                wm = wsbpool.tile([128, 4, 128], bf16, tag="wm",
                                  name=f"wm{b}")
                nc.vector.tensor_mul(
                    wm[:], w_ps[:],
                    maskh[:, None, :].broadcast_to([128, 4, 128]))
            return wm

        def emit_mfuse(b, wm):
            # M = Wp * BD(w) * Wv ; mt = M''^T = diag(a) M^T (bf16, [i, o])
            # cP = M b (+ Wp BD(w) bv) + bp
            aT, bvec, bvec_bf, wqf, wkf, cqf, ckf = ab_tiles[b]
            a_sb = wfpool.tile([128, 4, C], bf16, tag="wqf", name=f"a_sb{b}")
            mt_sb = mpool.tile([128, 4, C], bf16, tag="mt", name=f"mt{b}")
            cPfull = stpool.tile([128, 4], f32, tag="cPf", name=f"cPf{b}")
            with tc.tile_pool(name=f"m_ps{b}", bufs=2, space="PSUM") as mps, \
                 tc.tile_pool(name=f"c_ps{b}", bufs=2, space="PSUM") as cps:
                # A[c, i] = sum_d w[c, d] Wv[d, i]   (per 128-block m)
                for m in range(4):
                    a_ps = mps.tile([128, C], f32, tag="aps", name=f"aps{b}_{m}")
                    nc.tensor.matmul(a_ps[:], wm[:, m, :], wv_sb[:, m, :])
                    nc.scalar.activation(a_sb[:, m, :], a_ps[:], AF.Identity)
                # MT[i, o] = sum_c A[c, i] wpt[c, o]  -> M''T = a_i * MT
                for ni in range(4):
                    mt_ps = mps.tile([128, C], f32, tag="mtps", name=f"mtps{b}_{ni}")
                    for m in range(4):
                        nc.tensor.matmul(
                            mt_ps[:], a_sb[:, m, 128 * ni:128 * ni + 128],
                            wpt_sb[:, m, :],
                            start=(m == 0), stop=(m == 3))
                    nc.vector.tensor_scalar_mul(mt_sb[:, ni, :], mt_ps[:],
                                                aT[:, ni:ni + 1])
                # cP = M b: sum_i (a_i M[o,i]) * (b_i / a_i)
                rA = stpool.tile([128, 4], f32, tag="rA", name=f"rA{b}")
                nc.vector.reciprocal(rA[:], aT[:])
                bova = stpool.tile([128, 4], bf16, tag="bova", name=f"bova{b}")
                nc.vector.tensor_mul(bova[:], bvec[:], rA[:])
                cbv_sb = None
                if has_bv:
                    bvP_bf = stpool.tile([128, 4], bf16, tag="bvPbf",
                                         name=f"bvPbf{b}")
                    nc.vector.tensor_copy(bvP_bf[:], bvP[:])
                    cbv_ps = cps.tile([128, 4], f32, tag="cbvps", name=f"cbvps{b}")
                    for m in range(4):
                        nc.tensor.matmul(cbv_ps[:, m:m + 1], wm[:, m, :],
                                         bvP_bf[:, m:m + 1],
                                         start=(m == 0), stop=(m == 3),
                                         skip_group_check=True)
                    cbv_sb = stpool.tile([128, 4], bf16, tag="cbv",
                                         name=f"cbv{b}")
                    nc.vector.tensor_copy(cbv_sb[:], cbv_ps[:])
                cP_ps = cps.tile([128, 4], f32, tag="cPps", name=f"cPps{b}")
                n_mm = 4 * (4 + (4 if has_bv else 0))
                idx = 0
                for no in range(4):
                    for ji in range(4):
                        nc.tensor.matmul(
                            cP_ps[:, no:no + 1],
                            mt_sb[:, ji, 128 * no:128 * no + 128],
                            bova[:, ji:ji + 1],
                            start=(idx == 0), stop=(idx == n_mm - 1),
                            skip_group_check=True)
                        idx += 1
                    if has_bv:
                        for m in range(4):
                            nc.tensor.matmul(
                                cP_ps[:, no:no + 1],
                                wpt_sb[:, m, 128 * no:128 * no + 128],
                                cbv_sb[:, m:m + 1],
                                start=(idx == 0), stop=(idx == n_mm - 1),
                                skip_group_check=True)
                            idx += 1
                nc.vector.tensor_add(cPfull[:], cP_ps[:], bpP[:])
            return mt_sb, cPfull

        def emit_passB(b, mt_sb, cPfull):
            xbf = x_tiles[b]
            with ExitStack() as est_b:
                pj_pool = est_b.enter_context(
                    tc.tile_pool(name=f"pj_ps{b}", bufs=3, space="PSUM"))
                for i in range(NMi):
                    ot_h = [opool.tile([128, 2, TT], bf16, tag=f"ot{h}",
                                       name=f"ot{h}_{b}_{i}")
                            for h in range(2)]
                    for no in range(4):
                        pj = pj_pool.tile([128, TT], f32, tag="pj",
                                          name=f"pj_{b}_{i}_{no}")
                        for ji in range(4):
                            nc.tensor.matmul(
                                pj[:], mt_sb[:, ji, 128 * no:128 * no + 128],
                                xbf[:, ji, TT * i:TT * i + TT],
                                start=(ji == 0), stop=(ji == 3))
                        nc.vector.scalar_tensor_tensor(
                            ot_h[no // 2][:, no % 2, :], in0=pj[:],
                            scalar=cPfull[:, no:no + 1],
                            in1=xbf[:, no, TT * i:TT * i + TT],
                            op0=ALU.add, op1=ALU.add)
                    for h in range(2):
                        nc.sync.dma_start(
                            out_d[b, 256 * h:256 * h + 256,
                                  TT * i:TT * i + TT].rearrange(
                                "(j p) t -> p j t", p=128),
                            ot_h[h][:])

        # schedule: pass0(0); per batch: finalize, passA (next batch's
        # pass0 interleaved), M-fuse, passB.
        for i in range(NMi):
            emit_pass0_macro(0, i)
        for b in range(B):
            emit_finalize(b)
            wm = emit_passA(b, b + 1 if b + 1 < B else None)
            mt_sb, cPfull = emit_mfuse(b, wm)
            emit_passB(b, mt_sb, cPfull)

    nc.compile()
    return nc


def _to_part4(vec):
    # [512] -> [128, 4]: column j = channels 128j..128j+127
    return np.ascontiguousarray(vec.reshape(4, 128).T)


def _host_prep(x, gn_scale, gn_bias, wq, bq, wk, bk, wv, bv, wp, bp):
    import ml_dtypes
    bf = ml_dtypes.bfloat16
    sel = np.zeros((128, 8), dtype=np.float32)
    for p in range(128):
        sel[p, p // CG] = 1.0
    maskh = np.zeros((128, 128), dtype=np.float32)
    for p in range(128):
        maskh[p, (p // HC) * HC:(p // HC) * HC + HC] = 1.0
    consts = {
        "wqt": np.ascontiguousarray(wq.T).astype(bf),
        "wkt": np.ascontiguousarray(wk.T).astype(bf),
        "wv": np.ascontiguousarray(wv).astype(bf),
        "wpt": np.ascontiguousarray(wp.T).astype(bf),
        "gammaP": _to_part4(np.asarray(gn_scale)).astype(np.float32),
        "betaP": _to_part4(np.asarray(gn_bias)).astype(np.float32),
        "bq_row": np.asarray(bq).reshape(1, C).astype(np.float32),
        "bk_row": np.asarray(bk).reshape(1, C).astype(np.float32),
        "bvP": _to_part4(np.asarray(bv)).astype(np.float32),
        "bpP": _to_part4(np.asarray(bp)).astype(np.float32),
        "sel": sel,
        "selT": np.ascontiguousarray(sel.T),
        "ones1": np.ones((1, 128), dtype=np.float32).astype(bf),
        "maskh": maskh.astype(bf),
    }
    return consts


_NC_CACHE = {}


def kernel(x, gn_scale, gn_bias, wq, bq, wk, bk, wv, bv, wp, bp):
    from concourse.bass_utils import run_bass_kernel_spmd

    x = np.asarray(x, dtype=np.float32)
    consts = _host_prep(x, gn_scale, gn_bias, wq, bq, wk, bk, wv, bv, wp, bp)

    has_qk_bias = bool(np.any(np.asarray(bq)) or np.any(np.asarray(bk)))
    has_bv = bool(np.any(np.asarray(bv)))
    key = (B_SHARD, T_FULL, has_qk_bias, has_bv)
    if key not in _NC_CACHE:
        _NC_CACHE[key] = build_nc(B_SHARD, T_FULL, has_qk_bias=has_qk_bias,
                                  has_bv=has_bv)
    nc = _NC_CACHE[key]

    in_maps = []
    for c in range(N_CORES):
        m = dict(consts)
        m["x"] = np.ascontiguousarray(x[B_SHARD * c:B_SHARD * (c + 1)])
        in_maps.append(m)
    res = run_bass_kernel_spmd(nc, in_maps, core_ids=list(range(N_CORES)))
    out = np.concatenate([np.asarray(r["out"]).astype(np.float32)
                          for r in res.results], axis=0)
    return out
